# revision 22
# baseline (speedup 1.0000x reference)
"""Trainium2 Bass kernel for nn_AttnConvKernel (conv3x3 x2 -> unfold -> gram -> softmax).

Sharding: 8 cores = 4 batch samples x 2 H-halves. Each core computes both convs
for its half in a single fused matmul pass that directly produces the
[positions, channels] (transposed) layout needed by the attention contraction:
the 3x3-shifted x window is the stationary operand and [W1*scale | W2]
(128x384) is the moving operand, in float32r. Attention logits accumulate in
PSUM across the 64 patch-row tiles; a pairwise AllReduce sums the two
half-sample partials; softmax over (cin, 3x3) runs on-device.
"""

import numpy as np

B, CIN, COUT = 4, 128, 256
H = W = 384
WP = W // 3              # 128 patch columns
HALF_ROWS = H // 2       # 192
TILES = HALF_ROWS // 3   # 64 patch-rows per core
CH = CIN + COUT          # 384 fused output channels
NCORES = 8
SCALE = 1.0 / np.sqrt(CIN * 9)

_compiled = None
_runners = {}


def _build_nc(reps=1, act_split=True, lag=3, psum4=False):
    import concourse.mybir as mybir
    import concourse.tile as tile
    from concourse import bacc
    from concourse.masks import make_identity

    f32 = mybir.dt.float32
    f32r = mybir.dt.float32r

    nc = bacc.Bacc(target_bir_lowering=False, num_devices=NCORES)
    # x_half: rows [3t .. 3t+4] cover patch-row t with halo; cols pre-padded.
    x_half = nc.dram_tensor(
        "x_half", [CIN, HALF_ROWS + 2, W + 2], f32, kind="ExternalInput"
    )
    wcat = nc.dram_tensor("wcat", [CIN, 9, CH], f32, kind="ExternalInput")
    out_t = nc.dram_tensor("out", [COUT, CIN, 9], f32, kind="ExternalOutput")
    cc_in = nc.dram_tensor("cc_in", [CIN, 9 * COUT], f32)
    cc_out = nc.dram_tensor("cc_out", [CIN, 9 * COUT], f32)

    with tile.TileContext(nc) as tc:
        with (
            tc.tile_pool(name="xp", bufs=4 if psum4 else 3) as xp,
            tc.tile_pool(name="wp", bufs=1) as wp,
            tc.tile_pool(
                name="yp", bufs=(9 if psum4 else 6) if lag <= 2 else lag + 5
            ) as yp,
            tc.tile_pool(name="sp", bufs=1) as sp,
            tc.tile_pool(name="pc", bufs=4 if psum4 else 3, space="PSUM") as pc,
            tc.tile_pool(name="pa", bufs=1, space="PSUM") as pa,
        ):
            w_sb = wp.tile([CIN, 9, CH], f32r)
            nc.sync.dma_start(out=w_sb, in_=wcat[:, :, :].bitcast(f32r))

            # persistent attn logit accumulators: 2 k's per PSUM bank
            attn_ps = [
                pa.tile([CIN, 2, COUT], f32, tag=f"attn{i}", name=f"attn{i}")
                for i in range(4)
            ]
            if psum4:
                # k=8 accumulates in SBUF (frees a PSUM bank for the conv pool)
                acc8 = sp.tile([CIN, COUT], f32)
            else:
                attn_ps.append(
                    pa.tile([CIN, 1, COUT], f32, tag="attn4", name="attn4")
                )

            def attn_mm(k, yk, t):
                if psum4 and k == 8:
                    ps8 = pc.tile([CIN, COUT], f32, tag="conv", name="ps8")
                    nc.tensor.matmul(
                        ps8, yk[:, 0:CIN], yk[:, CIN:CH], start=True, stop=True
                    )
                    nc.vector.tensor_add(acc8, acc8, ps8)
                    return
                # start=True clears has_written for the WHOLE bank, so only the
                # first k of each 2-k bank may set it (at t=0). The second k's
                # first matmul overwrites via per-element has_written bits.
                nc.tensor.matmul(
                    attn_ps[k // 2][:, k % 2, :],
                    yk[:, 0:CIN],
                    yk[:, CIN:CH],
                    start=(t == 0 and k % 2 == 0),
                    stop=(t == TILES - 1),
                    skip_group_check=True,
                )

            for rep in range(reps):
                if psum4:
                    nc.vector.memset(acc8, 0.0)
                for t in range(TILES):
                    xt = xp.tile([CIN, 5, W + 2], f32r, name="xt")
                    nc.sync.dma_start(
                        out=xt, in_=x_half[:, 3 * t : 3 * t + 5, :].bitcast(f32r)
                    )
                    yks = []
                    for k in range(9):
                        kh, kw = divmod(k, 3)
                        ps = pc.tile([WP, CH], f32, tag="conv", name="ps")
                        for tap in range(9):
                            dh1, dw1 = divmod(tap, 3)
                            s = kw + dw1
                            lhsT = xt[:, kh + dh1, s : s + 3 * WP - 2 : 3]
                            nc.tensor.matmul(
                                ps,
                                lhsT,
                                w_sb[:, tap, :],
                                start=(tap == 0),
                                stop=(tap == 8),
                            )
                        yk = yp.tile([WP, CH], f32r, tag="y", name="yk")
                        if act_split and k % 2 == 0:
                            nc.scalar.copy(out=yk, in_=ps)
                        else:
                            nc.vector.tensor_copy(out=yk, in_=ps)
                        yks.append(yk)
                        # lag attn matmuls behind the drain copies so PE never
                        # waits on a copy
                        if k >= lag:
                            attn_mm(k - lag, yks[k - lag], t)
                    for k in range(9 - lag, 9):
                        attn_mm(k, yks[k], t)

            # ---- tail: merge pair halves, softmax, write out ----
            lg = sp.tile([CIN, 9 * COUT], f32)
            for i in range(4):
                nc.vector.tensor_copy(
                    out=lg[:, i * 512 : (i + 1) * 512], in_=attn_ps[i]
                )
            if psum4:
                nc.vector.tensor_copy(out=lg[:, 2048:2304], in_=acc8)
            else:
                nc.vector.tensor_copy(out=lg[:, 2048:2304], in_=attn_ps[4][:, 0, :])
            nc.sync.dma_start(out=cc_in[:, :], in_=lg)
            nc.gpsimd.collective_compute(
                "AllReduce",
                mybir.AluOpType.add,
                replica_groups=[[0, 1], [2, 3], [4, 5], [6, 7]],
                ins=[cc_in.ap().opt()],
                outs=[cc_out.ap().opt()],
            )
            lgs = sp.tile([CIN, 9, COUT], f32)
            nc.sync.dma_start(
                out=lgs, in_=cc_out[:, :].rearrange("p (k o) -> p k o", k=9)
            )

            ident = sp.tile([128, 128], f32)
            make_identity(nc, ident)
            soft = sp.tile([128, 2, CIN, 9], f32)
            for h in range(2):
                for k in range(9):
                    tp = pc.tile([128, 128], f32, tag="conv")
                    nc.tensor.transpose(
                        out=tp, in_=lgs[:, k, h * 128 : (h + 1) * 128], identity=ident
                    )
                    nc.vector.tensor_copy(out=soft[:, h, :, k], in_=tp)

            mx = sp.tile([128, 2], f32)
            nmx = sp.tile([128, 2], f32)
            sm = sp.tile([128, 2], f32)
            rs = sp.tile([128, 2], f32)
            for h in range(2):
                nc.vector.reduce_max(
                    out=mx[:, h : h + 1],
                    in_=soft[:, h],
                    axis=mybir.AxisListType.XY,
                )
                nc.scalar.mul(out=nmx[:, h : h + 1], in_=mx[:, h : h + 1], mul=-1.0)
                nc.scalar.activation(
                    out=soft[:, h],
                    in_=soft[:, h],
                    func=mybir.ActivationFunctionType.Exp,
                    bias=nmx[:, h : h + 1],
                    scale=1.0,
                    accum_out=sm[:, h : h + 1],
                )
                nc.vector.reciprocal(out=rs[:, h : h + 1], in_=sm[:, h : h + 1])
                nc.vector.tensor_scalar_mul(soft[:, h], soft[:, h], rs[:, h : h + 1])

            nc.sync.dma_start(
                out=out_t[:, :, :].rearrange("(h p) i k -> p h i k", h=2), in_=soft
            )
    nc.compile()
    return nc


def _prep_inputs(x, w1, w2):
    x = np.ascontiguousarray(np.asarray(x, dtype=np.float32))
    w1 = np.asarray(w1, dtype=np.float32)
    w2 = np.asarray(w2, dtype=np.float32)

    wcat = np.empty((CIN, 9, CH), np.float32)
    for dh in range(3):
        for dw in range(3):
            tap = dh * 3 + dw
            wcat[:, tap, :CIN] = w1[:, :, dh, dw].T * SCALE
            wcat[:, tap, CIN:] = w2[:, :, dh, dw].T

    xp = np.zeros((B, CIN, H + 2, W + 2), np.float32)
    xp[:, :, 1:-1, 1:-1] = x

    in_maps = []
    for c in range(NCORES):
        b, h = divmod(c, 2)
        xh = np.ascontiguousarray(xp[b, :, h * HALF_ROWS : h * HALF_ROWS + 194, :])
        in_maps.append({"x_half": xh, "wcat": wcat})
    return in_maps


class _Runner:
    """Compile once, execute many times with device-resident inputs."""

    def __init__(self, reps=1, **build_kw):
        import jax
        import concourse.mybir as mybir
        from concourse import bass2jax
        from jax.sharding import Mesh, PartitionSpec, NamedSharding
        from jax.experimental.shard_map import shard_map

        self.jax = jax
        nc = _build_nc(reps=reps, **build_kw)
        bass2jax.install_neuronx_cc_hook()

        partition_name = (
            nc.partition_id_tensor.name if nc.partition_id_tensor else None
        )
        in_names, out_names, out_avals, zero_outs = [], [], [], []
        for alloc in nc.m.functions[0].allocations:
            if not isinstance(alloc, mybir.MemoryLocationSet):
                continue
            name = alloc.memorylocations[0].name
            if alloc.kind == "ExternalInput":
                if name != partition_name:
                    in_names.append(name)
            elif alloc.kind == "ExternalOutput":
                out_names.append(name)
                shape = tuple(alloc.tensor_shape)
                dtype = mybir.dt.np(alloc.dtype)
                out_avals.append(jax.core.ShapedArray(shape, dtype))
                zero_outs.append(np.zeros(shape, dtype))
        n_params = len(in_names)
        n_outs = len(out_avals)
        all_names = in_names + out_names
        if partition_name is not None:
            all_names = all_names + [partition_name]

        def _body(*args):
            operands = list(args)
            if partition_name is not None:
                operands.append(bass2jax.partition_id_tensor())
            outs = bass2jax._bass_exec_p.bind(
                *operands,
                out_avals=tuple(out_avals),
                in_names=tuple(all_names),
                out_names=tuple(out_names),
                lowering_input_output_aliases=(),
                sim_require_finite=True,
                sim_require_nnan=True,
                nc=nc,
            )
            return tuple(outs)

        devices = jax.devices()[:NCORES]
        mesh = Mesh(np.asarray(devices), ("core",))
        self.sharded = jax.jit(
            shard_map(
                _body,
                mesh=mesh,
                in_specs=(PartitionSpec("core"),) * (n_params + n_outs),
                out_specs=(PartitionSpec("core"),) * n_outs,
                check_rep=False,
            ),
            keep_unused=True,
        )
        self.sharding = NamedSharding(mesh, PartitionSpec("core"))
        self.in_names = in_names
        self.out_names = out_names
        self.out_avals = out_avals
        self.dev_zeros = [
            jax.device_put(
                np.zeros((NCORES * z.shape[0], *z.shape[1:]), z.dtype), self.sharding
            )
            for z in zero_outs
        ]

    def put_inputs(self, in_maps):
        concat = [
            np.concatenate([np.asarray(m[name]) for m in in_maps], axis=0)
            for name in self.in_names
        ]
        return [self.jax.device_put(a, self.sharding) for a in concat]

    def execute(self, dev_inputs, n=1, block=True):
        for _ in range(n):
            out_arrs = self.sharded(*dev_inputs, *self.dev_zeros)
        if block:
            self.jax.block_until_ready(out_arrs)
        return out_arrs

    def run(self, in_maps):
        out_arrs = self.execute(self.put_inputs(in_maps))
        res = []
        for c in range(NCORES):
            res.append(
                {
                    name: np.asarray(out_arrs[i]).reshape(
                        NCORES, *self.out_avals[i].shape
                    )[c]
                    for i, name in enumerate(self.out_names)
                }
            )
        return res


def get_runner(reps=1, **build_kw):
    key = (reps, tuple(sorted(build_kw.items())))
    if key not in _runners:
        _runners[key] = _Runner(reps=reps, **build_kw)
    return _runners[key]


def kernel(x, w1, w2):
    in_maps = _prep_inputs(x, w1, w2)
    results = get_runner(reps=1).run(in_maps)
    out = np.empty((B, COUT, CIN, 9), np.float32)
    for b in range(B):
        out[b] = results[2 * b]["out"]
    return out
